# revision 7
# baseline (speedup 1.0000x reference)
"""Trainium2 Bass kernel for nn_DeChunkLayer (ragged EMA de-chunk).

Math (per batch row b):
    p[l]   = clip(boundary_prob[b, l, 1], EPS, 1-EPS)
    nb[l]  = cumsum_l(boundary_mask[b])          (>= 1 since l=0 is a boundary)
    h(k)   = (1-pb[k]) h(k-1) + pb[k] x[k]       (EMA over chunk rank k;
                                                  pb = p at the k-th boundary)
    out[l] = h(nb[l]-1)

Compact-scan + gather-expand design (v2):
  Only ~25% of positions are boundaries, so the EMA has only K ~= 2048
  distinct states and its inputs are x[0:K] read SEQUENTIALLY (the k-th
  EMA step uses x row k, not a gathered row).  Host precomputes the tiny
  per-row index arrays (cumsum nb, gather index idx[l] = nb[l]-1,
  compacted pb[k] = p at the k-th boundary, padded with 0 to K_max so
  padded steps are identity h = 1*h + 0*x).  Device then:
    phase A (compact scan over K_max ~= 2560 instead of 8192):
      per 512-k chunk: sequential DMA of x (bf16), bn = pb*x (DVE, f32),
      PE-transpose to [128_d, 512_k], DVE tensor_tensor_scan with f32
      carries chaining chunks, PE-transpose back, ACT downcast-copy to
      bf16 staging, one DMA to a DRAM scratch tensor hd[K_max, 512].
    phase B (expansion): out[l] = hd[idx[l]] via 64 indirect-DMA row
      gathers (offsets MUST be [128,1] - one per partition) landing in
      bf16 SBUF tiles DMA'd straight to the bf16 output.  Gather src APs
      are sliced hd[0:512*(c+1)] (valid since idx[l] <= l) so the Tile
      DRAM dependency tracker does not serialize later phase-A writes
      against earlier phase-B reads.
  Output is bf16 on device, upconverted to f32 on host (rel err ~2e-3,
  budget 2e-2).  HBM traffic/core: x 2.5MB + hd 2.5+8MB + out 8MB =
  21MB vs 32MB for the fused full-length scan; DVE scan work and PE
  transpose work drop 3.2x.

kernel(**inputs) takes FULL inputs, shards over 8 cores (4 batch rows x 2
D-halves), returns FULL (4, 8192, 1024) f32 output.
"""

import os
import sys

import numpy as np

sys.path.insert(0, "/opt/trn_rl_repo")

B, L, D = 4, 8192, 1024
NCORES = 8
DSH = D // 2          # 512 channels per core
NLT = L // 128        # 64 l-tiles of 128
NLC = L // 512        # 16 l-chunks of 512
NDT = DSH // 128      # 4 d-tiles of 128
EPS = 1e-4

_progs = {}  # K_max -> compiled Bass program


def _build_program(kmax):
    import concourse.bass as bass
    import concourse.mybir as mybir
    from concourse import bacc
    from concourse.bass import IndirectOffsetOnAxis
    from concourse.masks import make_identity
    from concourse.tile import TileContext

    f32 = mybir.dt.float32
    bf16 = mybir.dt.bfloat16
    i32 = mybir.dt.int32
    Op = mybir.AluOpType

    KC = kmax // 512      # k-chunks
    KT = kmax // 128      # k-tiles

    nc = bacc.Bacc("TRN2", target_bir_lowering=False, debug=False,
                   num_devices=NCORES)

    i16 = mybir.dt.int16
    x = nc.declare_dram_parameter("x", [kmax, DSH], bf16, isOutput=False)
    arow_d = nc.declare_dram_parameter("arow", [1, kmax], f32, isOutput=False)
    pbcm_d = nc.declare_dram_parameter("pbcm", [128, KT], f32, isOutput=False)
    # idx16: [128, NLC*32] int16; per l-chunk slice [:, 32c:32c+32] holds the
    # 512 gather indices wrapped 16-per-column (idx i at partition i%16,
    # col i//16), replicated 8x down the partitions for the 8 gpsimd cores.
    idx16_d = nc.declare_dram_parameter("idx16", [128, NLC * 32], i16,
                                        isOutput=False)
    out = nc.declare_dram_parameter("out", [L, DSH], bf16, isOutput=True)

    with TileContext(nc) as tc:
        with (
            tc.tile_pool(name="const", bufs=1) as cpool,
            tc.tile_pool(name="prep", bufs=1) as ppool,
            tc.tile_pool(name="hdp", bufs=1, space="DRAM") as hdp,
        ):
            ident = cpool.tile([128, 128], f32, tag="ident")
            make_identity(nc, ident[:])
            ones1 = cpool.tile([1, 128], f32, tag="ones1")
            nc.vector.memset(ones1[:], 1.0)

            arow = ppool.tile([1, kmax], f32, tag="arow")
            nc.sync.dma_start(out=arow[:], in_=arow_d[:])
            pbcm = ppool.tile([128, KT], f32, tag="pbcm")
            nc.sync.dma_start(out=pbcm[:], in_=pbcm_d[:])
            idx16 = ppool.tile([128, NLC * 32], i16, tag="idx16")
            nc.sync.dma_start(out=idx16[:], in_=idx16_d[:])

            hd = hdp.tile([kmax, DSH], bf16, tag="hd", name="hd")

            carries = [ppool.tile([128, 1], f32, tag=f"carry{t}",
                                  name=f"carry{t}")
                       for t in range(NDT)]

            with (
                tc.tile_pool(name="xg", bufs=4) as xgp,
                tc.tile_pool(name="bn", bufs=8) as bnp,
                tc.tile_pool(name="bt", bufs=5, space="PSUM") as btp,
                tc.tile_pool(name="apm", bufs=1, space="PSUM") as apmp,
                tc.tile_pool(name="asb", bufs=2) as asbp,
                tc.tile_pool(name="outT", bufs=8) as otp,
                tc.tile_pool(name="po", bufs=2, space="PSUM") as pop,
                tc.tile_pool(name="hst", bufs=2) as hstp,
                tc.tile_pool(name="gx", bufs=4) as gxp,
            ):
                def scan_chunk(c):
                    # broadcast a = (1-pb)[chunk] to 128 partitions
                    apm = apmp.tile([128, 512], f32, tag="apm",
                                    name=f"apm_{c}")
                    nc.tensor.matmul(
                        out=apm[:], lhsT=ones1[:],
                        rhs=arow[:][0:1, 512 * c:512 * (c + 1)],
                        start=True, stop=True)
                    a_sb = asbp.tile([128, 512], f32, tag="a_sb",
                                     name=f"asb_{c}")
                    nc.scalar.copy(out=a_sb[:], in_=apm[:])

                    # sequential x rows for this chunk (bf16)
                    xg4 = xgp.tile([128, 4 * DSH], bf16, tag="xg",
                                   name=f"xg_{c}")
                    for kt in range(4):
                        r0 = 512 * c + 128 * kt
                        nc.sync.dma_start(
                            out=xg4[:][:, DSH * kt:DSH * (kt + 1)],
                            in_=x[:][r0:r0 + 128, :])

                    bts = [btp.tile([128, 512], f32, tag="bt",
                                    name=f"bt{t}_{c}")
                           for t in range(NDT)]
                    for kt in range(4):
                        t_col = 4 * c + kt
                        bn = bnp.tile([128, DSH], f32, tag="bn",
                                      name=f"bn_{c}_{kt}")
                        nc.vector.tensor_scalar_mul(
                            bn[:], xg4[:][:, DSH * kt:DSH * (kt + 1)],
                            pbcm[:][:, t_col:t_col + 1])
                        for t in range(NDT):
                            nc.tensor.transpose(
                                out=bts[t][:][:, 128 * kt:128 * (kt + 1)],
                                in_=bn[:][:, 128 * t:128 * (t + 1)],
                                identity=ident[:])

                    outTs = [otp.tile([128, 512], f32, tag=f"outT{t}",
                                      name=f"outT{t}_{c}")
                             for t in range(NDT)]
                    for t in range(NDT):
                        nc.vector.tensor_tensor_scan(
                            out=outTs[t][:], data0=a_sb[:], data1=bts[t][:],
                            initial=(0.0 if c == 0
                                     else carries[t][:][:, 0:1]),
                            op0=Op.mult, op1=Op.add)
                        nc.vector.tensor_copy(carries[t][:][:, 0:1],
                                              outTs[t][:][:, 511:512])

                    # transpose back to [k, d] rows, downcast to bf16, store
                    hst = hstp.tile([128, 4 * DSH], bf16, tag="hst",
                                    name=f"hst_{c}")
                    for kt in range(4):
                        po = pop.tile([128, DSH], f32, tag="po",
                                      name=f"po_{c}_{kt}")
                        for t in range(NDT):
                            nc.tensor.transpose(
                                out=po[:][:, 128 * t:128 * (t + 1)],
                                in_=outTs[t][:][:, 128 * kt:128 * (kt + 1)],
                                identity=ident[:])
                        nc.scalar.copy(
                            out=hst[:][:, DSH * kt:DSH * (kt + 1)], in_=po[:])
                    nc.sync.dma_start(
                        out=hd[:][512 * c:512 * (c + 1), :].rearrange(
                            "(b a) d -> a b d", a=128),
                        in_=hst[:].rearrange("a (b d) -> a b d", b=4))

                def expand_chunk(c2):
                    # h rows needed by l-chunk c2 are < 512*(c2+1) since
                    # idx[l] <= l; slice the gather src so the DRAM dep
                    # tracker only orders against already-written chunks.
                    hi = min(512 * (c2 + 1), kmax)
                    g4 = gxp.tile([128, 4 * DSH], bf16, tag="gx",
                                  name=f"gx_{c2}")
                    nc.gpsimd.dma_gather(
                        out_ap=g4[:].rearrange("a (b d) -> a b d", b=4),
                        in_ap=hd[:][0:hi, :],
                        idxs_ap=idx16[:][:, 32 * c2:32 * (c2 + 1)],
                        num_idxs=512, num_idxs_reg=512, elem_size=DSH)
                    nc.sync.dma_start(
                        out=out[:][512 * c2:512 * (c2 + 1), :].rearrange(
                            "(b a) d -> a b d", a=128),
                        in_=g4[:].rearrange("a (b d) -> a b d", b=4))

                for c in range(KC):
                    scan_chunk(c)
                    expand_chunk(c)
                for c2 in range(KC, NLC):
                    expand_chunk(c2)

    nc.compile()
    return nc


def _install_profile_hook():
    """Provide antenv.axon_hooks (missing in this image) so
    run_bass_kernel_spmd(trace=True) can capture NTFF profiles via
    /opt/axon/libaxon_pjrt.so."""
    import sys as _sys
    import types
    import contextlib
    import ctypes

    if "antenv.axon_hooks" in _sys.modules:
        return
    try:
        lib = ctypes.CDLL("/opt/axon/libaxon_pjrt.so")
        if not hasattr(lib, "axon_start_nrt_profile"):
            return
    except OSError:
        return
    lib.axon_start_nrt_profile.argtypes = [
        ctypes.POINTER(ctypes.c_int64), ctypes.c_size_t]
    lib.axon_start_nrt_profile.restype = ctypes.c_int64
    lib.axon_stop_nrt_profile.argtypes = [ctypes.c_char_p]
    lib.axon_stop_nrt_profile.restype = ctypes.c_int64

    @contextlib.contextmanager
    def _hook(output_dir, device_ids):
        import jax
        jax.devices()
        if device_ids:
            ids = (ctypes.c_int64 * len(device_ids))(*device_ids)
            rc = lib.axon_start_nrt_profile(ids, len(device_ids))
        else:
            rc = lib.axon_start_nrt_profile(None, 0)
        if rc != 0:
            raise RuntimeError(f"axon_start_nrt_profile rc={rc}")
        try:
            yield
        finally:
            n = lib.axon_stop_nrt_profile(str(output_dir).encode())
            print(f"profile: {n} file(s) written to {output_dir}",
                  file=sys.stderr)

    m = types.ModuleType("antenv.axon_hooks")
    m.get_axon_ntff_profile_hook = lambda: _hook
    m.set_axon_ntff_profile_hook = lambda h: None
    _sys.modules["antenv.axon_hooks"] = m


def _get_program(kmax):
    if kmax not in _progs:
        _progs[kmax] = _build_program(kmax)
    return _progs[kmax]


def run(inputs, trace=False):
    """Returns (full_output, exec_time_ns or None)."""
    import ml_dtypes
    from concourse.bass_utils import run_bass_kernel_spmd

    bf16 = ml_dtypes.bfloat16
    hidden_states = np.asarray(inputs["hidden_states"], dtype=np.float32)
    boundary_mask = np.asarray(inputs["boundary_mask"]).astype(bool)
    boundary_prob = np.asarray(inputs["boundary_prob"], dtype=np.float32)

    # host index prep (tiny [B, L] arrays)
    p_full = np.clip(boundary_prob[:, :, 1], EPS, 1.0 - EPS)  # (B, L)
    nb = np.cumsum(boundary_mask, axis=1, dtype=np.int64)      # (B, L)
    idx_full = np.maximum(nb - 1, 0).astype(np.int32)          # (B, L)
    Ks = boundary_mask.sum(axis=1)                             # (B,)
    kmax = int(((int(Ks.max()) + 511) // 512) * 512)
    kmax = max(kmax, 512)
    KT = kmax // 128

    pb = np.zeros((B, kmax), np.float32)
    for b in range(B):
        pos = np.flatnonzero(boundary_mask[b])
        pb[b, :len(pos)] = p_full[b, pos]
    arow = (1.0 - pb).reshape(B, 1, kmax)                      # (B, 1, kmax)
    pbcm = np.ascontiguousarray(
        pb.reshape(B, KT, 128).transpose(0, 2, 1))             # (B, 128, KT)
    # dma_gather idx layout: idx i of chunk c at [i%16, 32c + i//16],
    # replicated 8x down partitions.
    idx16 = np.ascontiguousarray(np.tile(
        idx_full.reshape(B, NLC, 32, 16).transpose(0, 3, 1, 2)
        .reshape(B, 16, NLC * 32), (1, 8, 1))).astype(np.int16)  # (B,128,NLC*32)

    nc = _get_program(kmax)
    in_maps = []
    for c in range(NCORES):
        b, h = divmod(c, 2)
        in_maps.append({
            "x": np.ascontiguousarray(
                hidden_states[b, :kmax, h * DSH:(h + 1) * DSH]).astype(bf16),
            "arow": arow[b],
            "pbcm": pbcm[b],
            "idx16": idx16[b],
        })
    if trace:
        _install_profile_hook()
    res = run_bass_kernel_spmd(nc, in_maps, list(range(NCORES)), trace=trace)
    outs = res.results
    full = np.empty((B, L, D), np.float32)
    for c in range(NCORES):
        b, h = divmod(c, 2)
        full[b, :, h * DSH:(h + 1) * DSH] = outs[c]["out"].astype(np.float32)
    return full, res.exec_time_ns


def kernel(**inputs) -> np.ndarray:
    out, _ = run(inputs, trace=False)
    return out


# revision 8
# speedup vs baseline: 1.0231x; 1.0231x over previous
"""Trainium2 Bass kernel for nn_DeChunkLayer (ragged EMA de-chunk).

Math (per batch row b):
    p[l]   = clip(boundary_prob[b, l, 1], EPS, 1-EPS)
    nb[l]  = cumsum_l(boundary_mask[b])          (>= 1 since l=0 is a boundary)
    h(k)   = (1-pb[k]) h(k-1) + pb[k] x[k]       (EMA over chunk rank k;
                                                  pb = p at the k-th boundary)
    out[l] = h(nb[l]-1)

Compact-scan + gather-expand design (v2):
  Only ~25% of positions are boundaries, so the EMA has only K ~= 2048
  distinct states and its inputs are x[0:K] read SEQUENTIALLY (the k-th
  EMA step uses x row k, not a gathered row).  Host precomputes the tiny
  per-row index arrays (cumsum nb, gather index idx[l] = nb[l]-1,
  compacted pb[k] = p at the k-th boundary, padded with 0 to K_max so
  padded steps are identity h = 1*h + 0*x).  Device then:
    phase A (compact scan over K_max ~= 2560 instead of 8192):
      per 512-k chunk: sequential DMA of x (bf16), bn = pb*x (DVE, f32),
      PE-transpose to [128_d, 512_k], DVE tensor_tensor_scan with f32
      carries chaining chunks, PE-transpose back, ACT downcast-copy to
      bf16 staging, one DMA to a DRAM scratch tensor hd[K_max, 512].
    phase B (expansion): out[l] = hd[idx[l]] via 64 indirect-DMA row
      gathers (offsets MUST be [128,1] - one per partition) landing in
      bf16 SBUF tiles DMA'd straight to the bf16 output.  Gather src APs
      are sliced hd[0:512*(c+1)] (valid since idx[l] <= l) so the Tile
      DRAM dependency tracker does not serialize later phase-A writes
      against earlier phase-B reads.
  Output is bf16 on device, upconverted to f32 on host (rel err ~2e-3,
  budget 2e-2).  HBM traffic/core: x 2.5MB + hd 2.5+8MB + out 8MB =
  21MB vs 32MB for the fused full-length scan; DVE scan work and PE
  transpose work drop 3.2x.

kernel(**inputs) takes FULL inputs, shards over 8 cores (4 batch rows x 2
D-halves), returns FULL (4, 8192, 1024) f32 output.
"""

import os
import sys

import numpy as np

sys.path.insert(0, "/opt/trn_rl_repo")

B, L, D = 4, 8192, 1024
NCORES = 8
DSH = D // 2          # 512 channels per core
NLT = L // 128        # 64 l-tiles of 128
NLC = L // 512        # 16 l-chunks of 512
NDT = DSH // 128      # 4 d-tiles of 128
EPS = 1e-4

_progs = {}  # K_max -> compiled Bass program


def _build_program(kmax):
    import concourse.bass as bass
    import concourse.mybir as mybir
    from concourse import bacc
    from concourse.bass import IndirectOffsetOnAxis
    from concourse.masks import make_identity
    from concourse.tile import TileContext

    f32 = mybir.dt.float32
    bf16 = mybir.dt.bfloat16
    i32 = mybir.dt.int32
    Op = mybir.AluOpType

    KC = kmax // 512      # k-chunks
    KT = kmax // 128      # k-tiles

    nc = bacc.Bacc("TRN2", target_bir_lowering=False, debug=False,
                   num_devices=NCORES)

    i16 = mybir.dt.int16
    x = nc.declare_dram_parameter("x", [kmax, DSH], bf16, isOutput=False)
    arow_d = nc.declare_dram_parameter("arow", [1, kmax], f32, isOutput=False)
    pbcm_d = nc.declare_dram_parameter("pbcm", [128, KT], f32, isOutput=False)
    # idx16: [128, NLC*32] int16; per l-chunk slice [:, 32c:32c+32] holds the
    # 512 gather indices wrapped 16-per-column (idx i at partition i%16,
    # col i//16), replicated 8x down the partitions for the 8 gpsimd cores.
    idx16_d = nc.declare_dram_parameter("idx16", [128, NLC * 32], i16,
                                        isOutput=False)
    out = nc.declare_dram_parameter("out", [L, DSH], bf16, isOutput=True)

    with TileContext(nc) as tc:
        with (
            tc.tile_pool(name="const", bufs=1) as cpool,
            tc.tile_pool(name="prep", bufs=1) as ppool,
            tc.tile_pool(name="hdp", bufs=1, space="DRAM") as hdp,
        ):
            ident = cpool.tile([128, 128], f32, tag="ident")
            make_identity(nc, ident[:])
            ones1 = cpool.tile([1, 128], f32, tag="ones1")
            nc.vector.memset(ones1[:], 1.0)

            arow = ppool.tile([1, kmax], f32, tag="arow")
            nc.sync.dma_start(out=arow[:], in_=arow_d[:])
            pbcm = ppool.tile([128, KT], f32, tag="pbcm")
            nc.sync.dma_start(out=pbcm[:], in_=pbcm_d[:])
            idx16 = ppool.tile([128, NLC * 32], i16, tag="idx16")
            nc.sync.dma_start(out=idx16[:], in_=idx16_d[:])

            hd = hdp.tile([kmax, DSH], bf16, tag="hd", name="hd")

            carries = [ppool.tile([128, 1], f32, tag=f"carry{t}",
                                  name=f"carry{t}")
                       for t in range(NDT)]

            with (
                tc.tile_pool(name="xg", bufs=4) as xgp,
                tc.tile_pool(name="bn", bufs=8) as bnp,
                tc.tile_pool(name="bt", bufs=5, space="PSUM") as btp,
                tc.tile_pool(name="apm", bufs=1, space="PSUM") as apmp,
                tc.tile_pool(name="asb", bufs=2) as asbp,
                tc.tile_pool(name="outT", bufs=8) as otp,
                tc.tile_pool(name="po", bufs=2, space="PSUM") as pop,
                tc.tile_pool(name="hst", bufs=2) as hstp,
                tc.tile_pool(name="gx", bufs=4) as gxp,
            ):
                def scan_chunk(c):
                    # broadcast a = (1-pb)[chunk] to 128 partitions
                    apm = apmp.tile([128, 512], f32, tag="apm",
                                    name=f"apm_{c}")
                    nc.tensor.matmul(
                        out=apm[:], lhsT=ones1[:],
                        rhs=arow[:][0:1, 512 * c:512 * (c + 1)],
                        start=True, stop=True)
                    a_sb = asbp.tile([128, 512], f32, tag="a_sb",
                                     name=f"asb_{c}")
                    nc.scalar.copy(out=a_sb[:], in_=apm[:])

                    # sequential x rows for this chunk (bf16)
                    xg4 = xgp.tile([128, 4 * DSH], bf16, tag="xg",
                                   name=f"xg_{c}")
                    for kt in range(4):
                        r0 = 512 * c + 128 * kt
                        nc.sync.dma_start(
                            out=xg4[:][:, DSH * kt:DSH * (kt + 1)],
                            in_=x[:][r0:r0 + 128, :])

                    bts = [btp.tile([128, 512], f32, tag="bt",
                                    name=f"bt{t}_{c}")
                           for t in range(NDT)]
                    for kt in range(4):
                        t_col = 4 * c + kt
                        bn = bnp.tile([128, DSH], f32, tag="bn",
                                      name=f"bn_{c}_{kt}")
                        nc.vector.tensor_scalar_mul(
                            bn[:], xg4[:][:, DSH * kt:DSH * (kt + 1)],
                            pbcm[:][:, t_col:t_col + 1])
                        for t in range(NDT):
                            nc.tensor.transpose(
                                out=bts[t][:][:, 128 * kt:128 * (kt + 1)],
                                in_=bn[:][:, 128 * t:128 * (t + 1)],
                                identity=ident[:])

                    outTs = [otp.tile([128, 512], f32, tag=f"outT{t}",
                                      name=f"outT{t}_{c}")
                             for t in range(NDT)]
                    for t in range(NDT):
                        nc.vector.tensor_tensor_scan(
                            out=outTs[t][:], data0=a_sb[:], data1=bts[t][:],
                            initial=(0.0 if c == 0
                                     else carries[t][:][:, 0:1]),
                            op0=Op.mult, op1=Op.add)
                        nc.vector.tensor_copy(carries[t][:][:, 0:1],
                                              outTs[t][:][:, 511:512])

                    # transpose back to [k, d] rows, downcast to bf16, store
                    hst = hstp.tile([128, 4 * DSH], bf16, tag="hst",
                                    name=f"hst_{c}")
                    for kt in range(4):
                        po = pop.tile([128, DSH], f32, tag="po",
                                      name=f"po_{c}_{kt}")
                        for t in range(NDT):
                            nc.tensor.transpose(
                                out=po[:][:, 128 * t:128 * (t + 1)],
                                in_=outTs[t][:][:, 128 * kt:128 * (kt + 1)],
                                identity=ident[:])
                        nc.scalar.copy(
                            out=hst[:][:, DSH * kt:DSH * (kt + 1)], in_=po[:])
                    nc.sync.dma_start(
                        out=hd[:][512 * c:512 * (c + 1), :].rearrange(
                            "(b a) d -> a b d", a=128),
                        in_=hst[:].rearrange("a (b d) -> a b d", b=4))

                def expand_chunk(c2):
                    # h rows needed by l-chunk c2 are < 512*(c2+1) since
                    # idx[l] <= l; slice the gather src so the DRAM dep
                    # tracker only orders against already-written chunks.
                    hi = min(512 * (c2 + 1), kmax)
                    g4 = gxp.tile([128, 4 * DSH], bf16, tag="gx",
                                  name=f"gx_{c2}")
                    nc.gpsimd.dma_gather(
                        out_ap=g4[:].rearrange("a (b d) -> a b d", b=4),
                        in_ap=hd[:][0:hi, :],
                        idxs_ap=idx16[:][:, 32 * c2:32 * (c2 + 1)],
                        num_idxs=512, num_idxs_reg=512, elem_size=DSH)
                    nc.sync.dma_start(
                        out=out[:][512 * c2:512 * (c2 + 1), :].rearrange(
                            "(b a) d -> a b d", a=128),
                        in_=g4[:].rearrange("a (b d) -> a b d", b=4))

                # All scan chunks before all gathers: the DRAM dep tracker
                # is whole-tensor, so interleaving creates false WAR edges
                # (hd write of chunk c+1 waits on the gather of chunk c).
                for c in range(KC):
                    scan_chunk(c)
                for c2 in range(NLC):
                    expand_chunk(c2)

    nc.compile()
    return nc


def _install_profile_hook():
    """Provide antenv.axon_hooks (missing in this image) so
    run_bass_kernel_spmd(trace=True) can capture NTFF profiles via
    /opt/axon/libaxon_pjrt.so."""
    import sys as _sys
    import types
    import contextlib
    import ctypes

    if "antenv.axon_hooks" in _sys.modules:
        return
    try:
        lib = ctypes.CDLL("/opt/axon/libaxon_pjrt.so")
        if not hasattr(lib, "axon_start_nrt_profile"):
            return
    except OSError:
        return
    lib.axon_start_nrt_profile.argtypes = [
        ctypes.POINTER(ctypes.c_int64), ctypes.c_size_t]
    lib.axon_start_nrt_profile.restype = ctypes.c_int64
    lib.axon_stop_nrt_profile.argtypes = [ctypes.c_char_p]
    lib.axon_stop_nrt_profile.restype = ctypes.c_int64

    @contextlib.contextmanager
    def _hook(output_dir, device_ids):
        import jax
        jax.devices()
        if device_ids:
            ids = (ctypes.c_int64 * len(device_ids))(*device_ids)
            rc = lib.axon_start_nrt_profile(ids, len(device_ids))
        else:
            rc = lib.axon_start_nrt_profile(None, 0)
        if rc != 0:
            raise RuntimeError(f"axon_start_nrt_profile rc={rc}")
        try:
            yield
        finally:
            n = lib.axon_stop_nrt_profile(str(output_dir).encode())
            print(f"profile: {n} file(s) written to {output_dir}",
                  file=sys.stderr)

    m = types.ModuleType("antenv.axon_hooks")
    m.get_axon_ntff_profile_hook = lambda: _hook
    m.set_axon_ntff_profile_hook = lambda h: None
    _sys.modules["antenv.axon_hooks"] = m


def _get_program(kmax):
    if kmax not in _progs:
        _progs[kmax] = _build_program(kmax)
    return _progs[kmax]


def run(inputs, trace=False):
    """Returns (full_output, exec_time_ns or None)."""
    import ml_dtypes
    from concourse.bass_utils import run_bass_kernel_spmd

    bf16 = ml_dtypes.bfloat16
    hidden_states = np.asarray(inputs["hidden_states"], dtype=np.float32)
    boundary_mask = np.asarray(inputs["boundary_mask"]).astype(bool)
    boundary_prob = np.asarray(inputs["boundary_prob"], dtype=np.float32)

    # host index prep (tiny [B, L] arrays)
    p_full = np.clip(boundary_prob[:, :, 1], EPS, 1.0 - EPS)  # (B, L)
    nb = np.cumsum(boundary_mask, axis=1, dtype=np.int64)      # (B, L)
    idx_full = np.maximum(nb - 1, 0).astype(np.int32)          # (B, L)
    Ks = boundary_mask.sum(axis=1)                             # (B,)
    kmax = int(((int(Ks.max()) + 511) // 512) * 512)
    kmax = max(kmax, 512)
    KT = kmax // 128

    pb = np.zeros((B, kmax), np.float32)
    for b in range(B):
        pos = np.flatnonzero(boundary_mask[b])
        pb[b, :len(pos)] = p_full[b, pos]
    arow = (1.0 - pb).reshape(B, 1, kmax)                      # (B, 1, kmax)
    pbcm = np.ascontiguousarray(
        pb.reshape(B, KT, 128).transpose(0, 2, 1))             # (B, 128, KT)
    # dma_gather idx layout: idx i of chunk c at [i%16, 32c + i//16],
    # replicated 8x down partitions.
    idx16 = np.ascontiguousarray(np.tile(
        idx_full.reshape(B, NLC, 32, 16).transpose(0, 3, 1, 2)
        .reshape(B, 16, NLC * 32), (1, 8, 1))).astype(np.int16)  # (B,128,NLC*32)

    nc = _get_program(kmax)
    in_maps = []
    for c in range(NCORES):
        b, h = divmod(c, 2)
        in_maps.append({
            "x": np.ascontiguousarray(
                hidden_states[b, :kmax, h * DSH:(h + 1) * DSH]).astype(bf16),
            "arow": arow[b],
            "pbcm": pbcm[b],
            "idx16": idx16[b],
        })
    if trace:
        _install_profile_hook()
    res = run_bass_kernel_spmd(nc, in_maps, list(range(NCORES)), trace=trace)
    outs = res.results
    full = np.empty((B, L, D), np.float32)
    for c in range(NCORES):
        b, h = divmod(c, 2)
        full[b, :, h * DSH:(h + 1) * DSH] = outs[c]["out"].astype(np.float32)
    return full, res.exec_time_ns


def kernel(**inputs) -> np.ndarray:
    out, _ = run(inputs, trace=False)
    return out
